# revision 1
# baseline (speedup 1.0000x reference)
"""Trainium2 Bass kernel for the ExponentialEnvelopes module.

Math (per spin):
    feats[n,k]  = [charge, centered coords]           (nuclei features, [128, 4])
    Z[n,o]      = (feats @ W_pi)[n,o]                 (= zeta.T)
    P[n,o]      = (feats @ W_zeta)[n,o]               (= pi.T)
    d[e,n]      = ||e_coords[e] - nuc_coords[n]||
    orb[e,o]    = sum_n P[n,o] * exp(-d[e,n] * |Z[n,o]|)
    out[s,det,e,me] = orb reshaped

All masks are all-ones for this problem (spec fill="ones"), so the masked
branches of the reference collapse to the above.

Sharding: electrons are sharded across the 8 cores (16 electrons/core, both
spins), orbitals (4096) are kept whole per core.  This gives each exp
activation instruction a 4096-wide free dim with the per-electron distance
applied through the ACT engine's per-partition `scale` operand, so the
[nuclei x orbital] outer product inside the exponent costs zero extra
instructions.  Per electron (steady state, ACT-bound at ~3.7us/electron):
    ACT : T = exp(absZ * (-d[:,e]))       [128, 4096] fp16   (~3.7us)
    DVE : T *= piT (in place, quarters)   [128, 4096] fp16   (~2.8us)
    PE  : 8x matmul(lhsT=onehot[128,16], rhs=T chunk [128,512])
          -> accumulates the partition-reduction into PSUM row e of the
          per-chunk [16, 512] accumulator (start at e=0, stop at e=15)
Host gathers the per-core [2, 16, 4096] slabs (already in orb layout).

Measured: ~156us HW exec on 8 cores (scalar-engine exp floor is ~119us;
NEFF preamble + zeta/pi setup head ~15us, drain/barrier tail ~20us).
"""

import numpy as np
from contextlib import ExitStack

NE = 128          # electrons per spin (total)
NN = 128          # nuclei
NDET = 32
NORB = 4096       # n_det * max_e
N_CORES = 8
E_PER_CORE = NE // N_CORES   # 16
NBLK = NORB // 128
WBLK = NORB // 512           # zeta/pi matmul blocks of 512

_CACHE = {}

LAST_RESULTS = None  # BassKernelResults of the most recent run (for test harness)


def _split_multiwaits(nc, blocks):
    """Every TPB engine instruction has exactly ONE embedded sync-wait slot
    (NEURON_ISA_TPB_EVENTS); Tile's sem assignment can emit several waits on
    one instruction, which walrus rejects ("Too many sync wait commands").
    Hoist all but the last wait onto fresh single-wait NOPs inserted just
    before the instruction on the same engine stream."""
    from concourse import mybir

    for bb, insts in blocks.items():
        out = []
        changed = False
        for inst in insts:
            si = getattr(inst, "sync_info", None)
            waits = list(si.on_wait) if si is not None and si.on_wait else []
            if len(waits) > 1:
                for w in waits[:-1]:
                    nop = mybir.InstNoOp(
                        name=nc.get_next_instruction_name(), ins=[], outs=[])
                    nop.engine = inst.engine
                    nop.sync_info = mybir.SyncInfo(on_wait=[w], on_update=[])
                    out.append(nop)
                inst.sync_info = mybir.SyncInfo(
                    on_wait=[waits[-1]], on_update=list(si.on_update))
                changed = True
            out.append(inst)
        if changed:
            insts[:] = out


def _build_module():
    import concourse.bass as bass
    import concourse.tile as tile
    from concourse import mybir
    from concourse.alu_op_type import AluOpType

    class FixupTileContext(tile.TileContext):
        def _lower_ordered_insts(self, postordered_blocks):
            _split_multiwaits(self.nc, postordered_blocks)
            return super()._lower_ordered_insts(postordered_blocks)

        def _drain_and_barrier(self, tick_clock, wait_clock):
            # The kernel-tail drain waits on the full global clock (~11 sems),
            # over the single embedded wait slot.  Pre-observe the clock on
            # the sync engine via single-wait NOPs; add_sem_waits then elides
            # the (now redundant) waits on the real drain.
            from concourse.vector_clock import ScopedClock

            probe = self.nc.sync.nop()
            wait_clock.add_sem_waits(
                probe.ins, ScopedClock({None: tick_clock.global_clock}))
            si = probe.ins.sync_info
            waits = list(si.on_wait) if si is not None and si.on_wait else []
            if len(waits) > 1:
                probe.ins.sync_info = mybir.SyncInfo(
                    on_wait=[waits[0]], on_update=list(si.on_update or []))
                for w in waits[1:]:
                    extra = self.nc.sync.nop()
                    extra.ins.sync_info = mybir.SyncInfo(
                        on_wait=[w], on_update=[])
            ret = super()._drain_and_barrier(tick_clock, wait_clock)
            # The probes above pre-observed the whole clock on SP in program
            # order, so the tail drain's own waits are redundant — and exceed
            # the single embedded wait slot.  Strip them.
            for blk in self.nc.m.functions[0].blocks:
                for i in blk.instructions:
                    si = getattr(i, "sync_info", None)
                    if (isinstance(i, mybir.InstDrain) and si is not None
                            and si.on_wait and len(si.on_wait) > 1):
                        i.sync_info = mybir.SyncInfo(
                            on_wait=[], on_update=list(si.on_update or []))
            return ret

    f32 = mybir.dt.float32
    f16 = mybir.dt.float16
    AF = mybir.ActivationFunctionType
    AX = mybir.AxisListType.X
    E = E_PER_CORE

    nc = bass.Bass(trn_type="TRN2")

    # all small inputs packed into one DMA: [3, 288] =
    #   [:, 0:128] nucT rows, [0, 128:256] charges, [:, 256:272] eT_up,
    #   [:, 272:288] eT_dn  (all slices start at partition 0)
    d_small = nc.dram_tensor("small", [3, 2 * NN + 2 * E], f32,
                             kind="ExternalInput")
    # W matrices pre-split by the host into charge rows (k=0) and coord rows
    # (k=1..3) so every SBUF access pattern starts at partition 0; all four
    # matrices are packed along the free dim: index (s, m) at (2*s+m)*NORB.
    d_w4 = nc.dram_tensor("w4", [4, 4 * NORB], f16, kind="ExternalInput")
    # per-core output slab: [spin][e_local][orbital] (directly in orb layout)
    d_out = nc.dram_tensor("out", [2, E, NORB], f32, kind="ExternalOutput")

    with ExitStack() as ctx:
        tc = ctx.enter_context(FixupTileContext(nc))
        const = ctx.enter_context(tc.tile_pool(name="const", bufs=1))
        wpool = ctx.enter_context(tc.tile_pool(name="wload", bufs=1))
        tpool = ctx.enter_context(tc.tile_pool(name="texp", bufs=5))
        opool = ctx.enter_context(tc.tile_pool(name="outsb", bufs=8))
        psum = ctx.enter_context(tc.tile_pool(name="ps", bufs=1, space="PSUM"))
        # round-robin bank tags for transient setup psum tiles
        _bk = [0]

        def ps_tile(shape, tag=None):
            if tag is None:
                tag = f"bk{_bk[0] % 8}"
            _bk[0] += 1
            return psum.tile(shape, f32, tag=tag, name=f"ps{_bk[0]}_{tag}")

        # ---------------- small loads (single DMA) ----------------
        s_small = const.tile([3, 2 * NN + 2 * E], f32, tag="small")
        nc.sync.dma_start(s_small[:], d_small[:])
        s_nucT = s_small[:, 0:NN]
        s_chg = s_small[0:1, NN:2 * NN]
        s_eT = [s_small[:, 2 * NN:2 * NN + E],
                s_small[:, 2 * NN + E:2 * NN + 2 * E]]
        s_cnuc = const.tile([3, NN], f32, tag="cnuc")  # centered coords
        nc.vector.tensor_copy(s_cnuc[:], s_nucT)

        # W quarter 0 immediately (spin0-zeta needs it first; no deps)
        s_w4 = wpool.tile([4, 4 * NORB], f16, tag="w4")
        nc.sync.dma_start(s_w4[:, 0:NORB], d_w4[:, 0:NORB])

        # masked mean-centering of nuclear coords (mask all ones -> count=NN)
        s_mean = const.tile([3, 1], f32, tag="mean")
        nc.vector.tensor_reduce(s_mean[:], s_cnuc[:], AX, AluOpType.add)
        nc.vector.tensor_scalar_mul(s_mean[:], s_mean[:], 1.0 / NN)
        nc.vector.tensor_scalar(s_cnuc[:], s_cnuc[:],
                                s_mean[:, 0:1], None, AluOpType.subtract)

        # pieces for d2[n,e] = |n|^2 + |e|^2 - 2 n.e  (3 accumulating matmuls)
        s_m2n = const.tile([3, NN], f32, tag="m2n")
        nc.vector.tensor_scalar_mul(s_m2n[:], s_nucT, -2.0)
        s_nsq = const.tile([3, NN], f32, tag="nsq")
        nc.vector.tensor_mul(s_nsq[:], s_nucT, s_nucT)
        s_ones3 = const.tile([3, 1], f32, tag="ones3")
        nc.vector.memset(s_ones3[:], 1.0)
        s_onesrow = const.tile([1, NN], f32, tag="onesrow")
        nc.vector.memset(s_onesrow[:], 1.0)

        ps_n2 = ps_tile([1, NN], tag="bk0")
        nc.tensor.matmul(ps_n2[:], lhsT=s_ones3[:], rhs=s_nsq[:],
                         start=True, stop=True)
        s_n2 = const.tile([1, NN], f32, tag="n2")
        nc.vector.tensor_copy(s_n2[:], ps_n2[:])

        s_negd = []
        for s in (0, 1):
            s_esq = const.tile([3, E], f32, tag=f"esq{s}")
            nc.vector.tensor_mul(s_esq[:], s_eT[s], s_eT[s])
            ps_e2 = ps_tile([1, E], tag="bk1")
            nc.tensor.matmul(ps_e2[:], lhsT=s_ones3[:], rhs=s_esq[:],
                             start=True, stop=True)
            s_e2 = const.tile([1, E], f32, tag=f"e2{s}")
            nc.vector.tensor_copy(s_e2[:], ps_e2[:])

            ps_d2 = ps_tile([NN, E], tag="bk2")
            nc.tensor.matmul(ps_d2[:], lhsT=s_m2n[:], rhs=s_eT[s],
                             start=True, stop=False)
            nc.tensor.matmul(ps_d2[:], lhsT=s_n2[:], rhs=s_onesrow[:, 0:E],
                             start=False, stop=False)
            nc.tensor.matmul(ps_d2[:], lhsT=s_onesrow[:], rhs=s_e2[:],
                             start=False, stop=True)
            nd = const.tile([NN, E], f32, tag=f"negd{s}")
            # d = exp(0.5*ln(d2)): stays inside the natural_log_exp table
            # set (sqrt would force a second ACT table load + switch).
            # Guard: the expansion |n|^2+|e|^2-2n.e can round negative for
            # near-coincident points; clamp before Ln.
            s_d2c = const.tile([NN, E], f32, tag=f"d2c{s}")
            nc.vector.tensor_scalar_max(s_d2c[:], ps_d2[:], 1e-24)
            nc.scalar.activation(nd[:], s_d2c[:], AF.Ln)
            nc.scalar.activation(nd[:], nd[:], AF.Exp, scale=0.5)
            nc.vector.tensor_scalar_mul(nd[:], nd[:], -1.0)
            s_negd.append(nd)

        # fp16 feats tile [4, NN] = [charge; centered coords] for K=4 matmuls.
        # Rows 1..3 are placed by DMA (engines cannot write partition base 1).
        s_chg16 = const.tile([1, NN], f16, tag="chg16")
        nc.vector.tensor_copy(s_chg16[:], s_chg)
        s_cnuc16 = const.tile([3, NN], f16, tag="cnuc16")
        nc.vector.tensor_copy(s_cnuc16[:], s_cnuc[:])
        s_f16 = const.tile([4, NN], f16, tag="feats16")
        nc.sync.dma_start(s_f16[0:1, :], s_chg16[:])
        nc.sync.dma_start(s_f16[1:4, :], s_cnuc16[:])
        # remaining W quarters after the feats assembly DMAs
        for q in range(1, 4):
            qs = slice(q * NORB, (q + 1) * NORB)
            nc.sync.dma_start(s_w4[:, qs], d_w4[:, qs])

        # ---------------- zeta / pi ----------------
        s_absz = []
        s_piT = []
        for s in (0, 1):
            s_absz.append(const.tile([128, NORB], f32, tag=f"absz{s}",
                                     name=f"absz{s}"))
            s_piT.append(const.tile([128, NORB], f16, tag=f"pit{s}",
                                    name=f"pit{s}"))

        def w_matmul(dst_ps, w_off, blk):
            sl = slice(w_off + blk * 512, w_off + (blk + 1) * 512)
            nc.tensor.matmul(dst_ps[:], lhsT=s_f16[:], rhs=s_w4[:, sl],
                             start=True, stop=True)

        def emit_zeta_blk(s, blk):
            sl = slice(blk * 512, (blk + 1) * 512)
            ps_z = ps_tile([128, 512])
            w_matmul(ps_z, (2 * s) * NORB, blk)       # zeta uses W_pi
            # |z|: for spin0 alternate engines so the 8-op chain halves in
            # latency (ACT is idle before the exp stream starts); spin1's
            # abs must stay off ACT (it runs during spin0's exp stream)
            if s == 1 or blk % 2 == 0:
                nc.vector.tensor_scalar(
                    s_absz[s][:, sl].bitcast(mybir.dt.uint32),
                    ps_z[:].bitcast(mybir.dt.uint32),
                    0x7FFFFFFF, None, AluOpType.bitwise_and)
            else:
                nc.scalar.activation(s_absz[s][:, sl], ps_z[:], AF.Abs)

        def emit_pi_blk(s, blk):
            sl = slice(blk * 512, (blk + 1) * 512)
            ps_p = ps_tile([128, 512])
            w_matmul(ps_p, (2 * s + 1) * NORB, blk)   # pi uses W_zeta
            nc.vector.tensor_copy(s_piT[s][:, sl], ps_p[:])

        # One-hot is needed before the hoisted first-exp below
        s_oh0 = None

        # spin0's zeta first (needed to start its exp stream), then the rest.
        # The very first exp's halves are emitted INSIDE the zeta loop so the
        # strict ACT FIFO doesn't queue them behind later abs ops.
        t_exp0 = tpool.tile([128, NORB], f16, tag="T", name="texp_e0")
        Hh = NORB // 2
        for blk in range(WBLK):
            emit_zeta_blk(0, blk)
            if blk == 3:
                nc.scalar.activation(t_exp0[:, 0:Hh], s_absz[0][:, 0:Hh],
                                     AF.Exp, scale=s_negd[0][:, 0:1])
        nc.scalar.activation(t_exp0[:, Hh:], s_absz[0][:, Hh:],
                             AF.Exp, scale=s_negd[0][:, 0:1])
        for blk in range(WBLK):
            emit_pi_blk(0, blk)
        for blk in range(WBLK):
            emit_zeta_blk(1, blk)
        for blk in range(WBLK):
            emit_pi_blk(1, blk)

        # One-hot selector: lhsT slice e is [128, E] with column e all-ones.
        # The reduce matmul then computes out[m,o] = sum_n (m==e) * PT[n,o],
        # i.e. the partition-reduction lands on PSUM row e; electrons
        # accumulate into the same [E, 512] bank via start/stop groups.
        # LDWEIGHTS of an [128, E] slice is ~E cycles vs 128 for PT-stationary.
        s_oh = const.tile([128, E * E], f16, tag="onehot")
        nc.vector.memset(s_oh[:], 0.0)
        for e in range(E):
            nc.vector.memset(s_oh[:, e * E + e:e * E + e + 1], 1.0)

        # ---------------- main loop ----------------
        NCHUNK = NORB // 512   # 8 psum banks, one per 512-orbital chunk
        for s in (0, 1):
            ps_orb = [ps_tile([E, 512], tag=f"bk{c}") for c in range(NCHUNK)]
            for e in range(E):
                if s == 0 and e == 0:
                    t_exp = t_exp0   # activations already emitted (hoisted)
                elif s == 1 and (e == 0 or e == E - 1):
                    # split spin1's first exp (smooths the spin boundary) and
                    # the last exp (tail TT/matmuls/evac start half earlier)
                    t_exp = tpool.tile([128, NORB], f16, tag="T")
                    Hh = NORB // 2
                    nc.scalar.activation(t_exp[:, 0:Hh],
                                         s_absz[s][:, 0:Hh], AF.Exp,
                                         scale=s_negd[s][:, e:e + 1])
                    nc.scalar.activation(t_exp[:, Hh:],
                                         s_absz[s][:, Hh:], AF.Exp,
                                         scale=s_negd[s][:, e:e + 1])
                else:
                    t_exp = tpool.tile([128, NORB], f16, tag="T")
                    nc.scalar.activation(t_exp[:], s_absz[s][:], AF.Exp,
                                         scale=s_negd[s][:, e:e + 1])
                # in-place pi-weighting: t_exp *= piT (quarters so the
                # first reduce matmuls start after 1/4 of the multiply)
                H = NORB // 4
                for h in range(4):
                    nc.vector.tensor_mul(t_exp[:, h * H:(h + 1) * H],
                                         t_exp[:, h * H:(h + 1) * H],
                                         s_piT[s][:, h * H:(h + 1) * H])
                for c in range(NCHUNK):
                    nc.tensor.matmul(ps_orb[c][:],
                                     lhsT=s_oh[:, e * E:(e + 1) * E],
                                     rhs=t_exp[:, c * 512:(c + 1) * 512],
                                     start=(e == 0), stop=(e == E - 1))
            for c in range(NCHUNK):
                s_o = opool.tile([E, 512], f32, tag="osb")
                if c % 2 == 0:
                    nc.vector.tensor_copy(s_o[:], ps_orb[c][:])
                else:
                    nc.scalar.copy(s_o[:], ps_orb[c][:])
                # spread the final evac DMAs across two queues
                dma_eng = nc.gpsimd if (s == 1 and c % 2 == 1) else nc.sync
                dma_eng.dma_start(d_out[s][:, c * 512:(c + 1) * 512], s_o[:])

    return nc


def _get_module():
    if "nc" not in _CACHE:
        _CACHE["nc"] = _build_module()
    return _CACHE["nc"]


def kernel(**inputs) -> np.ndarray:
    global LAST_RESULTS
    nc = _get_module()
    from concourse.bass_utils import run_bass_kernel_spmd

    up = np.ascontiguousarray(np.asarray(inputs["up_coords"], dtype=np.float32))
    down = np.ascontiguousarray(np.asarray(inputs["down_coords"], dtype=np.float32))
    nuc = np.asarray(inputs["nuc_coords"], dtype=np.float32)
    chg = np.asarray(inputs["nuc_charges"], dtype=np.float32)
    w = {
        k: np.ascontiguousarray(np.asarray(inputs[k], dtype=np.float32))
        for k in ("W_pi_up", "W_zeta_up", "W_pi_down", "W_zeta_down")
    }
    nucT = nuc.T                                  # [3, 128]

    worder = ("W_pi_up", "W_zeta_up", "W_pi_down", "W_zeta_down")
    wsplit = {
        "w4": np.ascontiguousarray(np.concatenate(
            [w[n] for n in worder], axis=1).astype(np.float16)),
    }

    in_maps = []
    for c in range(N_CORES):
        sl = slice(c * E_PER_CORE, (c + 1) * E_PER_CORE)
        small = np.zeros((3, 2 * NN + 2 * E_PER_CORE), dtype=np.float32)
        small[:, 0:NN] = nucT
        small[0, NN:2 * NN] = chg
        small[:, 2 * NN:2 * NN + E_PER_CORE] = up[sl].T
        small[:, 2 * NN + E_PER_CORE:] = down[sl].T
        in_maps.append({"small": small, **wsplit})

    res = run_bass_kernel_spmd(nc, in_maps, core_ids=list(range(N_CORES)))
    LAST_RESULTS = res

    # gather: per-core slab is already [2, e_local, orbital]
    orb = np.empty((2, NE, NORB), dtype=np.float32)
    for c in range(N_CORES):
        a = np.asarray(res.results[c]["out"])            # [2, E, NORB]
        orb[:, c * E_PER_CORE:(c + 1) * E_PER_CORE, :] = a

    # [2, n_e, n_det*max_e] -> [2, n_det, n_e, max_e]
    out = orb.reshape(2, NE, NDET, NE).swapaxes(1, 2)
    return np.ascontiguousarray(out)

